# revision 14
# baseline (speedup 1.0000x reference)
"""CenterLoss kernel for Trainium2, 8-core SPMD.

loss = mean_i( 0.5 * || x[i] - centers[labels[i]] ||^2 )

Sharding: data-parallel over the batch. Each of the 8 cores gets 128
samples (x rows + labels) and a replicated view of the full centers
table in its HBM. On-device, each core indirect-DMA-gathers only the
128 label-matched center rows (256 KB) — never touching the other
64 MB of the table — forms the diff in the DMA datapath itself, runs a
fused square + row-sum on the scalar engine, and reduces across partitions
with a ones-vector matmul on the tensor engine. Each core emits one
partial sum; the host combines 8 scalars (the unshard step) and
applies the 0.5/N scaling.

Written in raw bacc (manual semaphores, no TileContext): the linear
dependency chain needs only 5 sems, and skipping Tile's exit barriers
saves ~1.5us of epilogue. The NEFF epilogue's own drain flushes the
final output DMA, so no explicit wait on it is needed.

The subtraction is fused into the gather via the SDMA datapath's
inline CCE ALU: the host stages -x, and the indirect gather RMW-adds
the center rows onto it (compute_op=add), leaving c - x in SBUF with
no vector-engine pass. (c-x)^2 == (x-c)^2, so the sign is free.
"""

import numpy as np

N_CORES = 8
BATCH = 1024
FEAT = 512
NUM_CLASSES = 32768
B_LOC = BATCH // N_CORES  # 128 == SBUF partition count

_compiled = None


def _build():
    import concourse.bacc as bacc
    import concourse.bass as bass
    import concourse.mybir as mybir

    f32 = mybir.dt.float32
    i32 = mybir.dt.int32

    nc = bacc.Bacc(
        "TRN2",
        target_bir_lowering=False,
        debug=False,
        enable_asserts=False,
        num_devices=N_CORES,
    )

    x_d = nc.dram_tensor("x", [B_LOC, FEAT], f32, kind="ExternalInput").ap()
    lab_d = nc.dram_tensor("labels", [B_LOC, 1], i32, kind="ExternalInput").ap()
    cen_d = nc.dram_tensor(
        "centers", [NUM_CLASSES, FEAT], f32, kind="ExternalInput"
    ).ap()
    out_d = nc.dram_tensor("out", [1, 1], f32, kind="ExternalOutput").ap()

    with (
        nc.sbuf_tensor("xt", [B_LOC, FEAT], f32) as xt,
        nc.sbuf_tensor("lab", [B_LOC, 1], i32) as lab,
        nc.sbuf_tensor("sq", [B_LOC, FEAT], f32) as sq,
        nc.sbuf_tensor("row", [B_LOC, 1], f32) as row,
        nc.sbuf_tensor("ones", [B_LOC, 1], f32) as ones,
        nc.sbuf_tensor("res", [1, 1], f32) as res,
        nc.psum_tensor("acc", [1, 1], f32) as acc,
        nc.semaphore("lsem") as lsem,
        nc.semaphore("xsem") as xsem,
        nc.semaphore("gsem") as gsem,
        nc.semaphore("osem") as osem,
        nc.semaphore("csem") as csem,
        nc.semaphore("msem") as msem,
    ):
        # labels head the critical chain (labels -> gather); -x goes on
        # the scalar HWDGE queue so the two issue in parallel.
        nc.sync.dma_start(lab.ap(), lab_d).then_inc(lsem, 16)
        nc.scalar.dma_start(xt.ap(), x_d).then_inc(xsem, 16)
        nc.vector.memset(ones.ap(), 1.0).then_inc(msem, 1)

        # Gather RMW: xt := centers[lab] + (-x) = c - x via the CCE ALU.
        nc.gpsimd.wait_ge(lsem, 16)
        nc.gpsimd.wait_ge(xsem, 16)
        nc.gpsimd.indirect_dma_start(
            out=xt.ap(),
            out_offset=None,
            in_=cen_d,
            in_offset=bass.IndirectOffsetOnAxis(ap=lab.ap()[:, :1], axis=0),
            compute_op=mybir.AluOpType.add,
        ).then_inc(gsem, 16)

        # One ACT op: sq = (c-x)^2 AND row = sum_free(sq)
        nc.scalar.wait_ge(gsem, 16)
        nc.scalar.activation(
            out=sq.ap(),
            in_=xt.ap(),
            func=mybir.ActivationFunctionType.Square,
            accum_out=row.ap(),
        ).then_inc(csem, 1)

        # Partition reduce on PE: ones[128,1].T @ row[128,1] -> [1,1].
        # Keeps the output store a single 4B write (a [128,1] store is
        # 128 scattered 4B descriptors and stalls the final drain ~8us).
        nc.tensor.wait_ge(msem, 1)
        nc.tensor.wait_ge(csem, 1)
        nc.tensor.matmul(
            out=acc.ap(), lhsT=ones.ap(), rhs=row.ap(), start=True, stop=True
        ).then_inc(csem, 1)

        nc.vector.wait_ge(csem, 2)
        nc.vector.tensor_copy(out=res.ap(), in_=acc.ap()).then_inc(csem, 1)

        nc.sync.wait_ge(csem, 3)
        # No wait on osem: the NEFF epilogue drain flushes this queue.
        nc.sync.dma_start(out_d, res.ap()).then_inc(osem, 16)

    nc.compile()
    return nc


def _get_compiled():
    global _compiled
    if _compiled is None:
        _compiled = _build()
    return _compiled


def _in_maps(x, labels, centers):
    # stage -x: the gather's CCE add then leaves c - x in SBUF
    xs = np.ascontiguousarray(-np.asarray(x, dtype=np.float32)).reshape(
        N_CORES, B_LOC, FEAT
    )
    lab32 = np.ascontiguousarray(
        np.asarray(labels).astype(np.int32).reshape(N_CORES, B_LOC, 1)
    )
    cen = np.ascontiguousarray(np.asarray(centers, dtype=np.float32))
    return [
        {"x": xs[i], "labels": lab32[i], "centers": cen} for i in range(N_CORES)
    ]


def kernel(x, labels, centers):
    from concourse.bass_utils import run_bass_kernel_spmd

    nc = _get_compiled()
    res = run_bass_kernel_spmd(nc, _in_maps(x, labels, centers), list(range(N_CORES)))
    partials = np.array(
        [np.float64(r["out"].reshape(())) for r in res.results], dtype=np.float64
    )
    total = 0.5 * partials.sum() / BATCH
    return np.asarray(total, dtype=np.float32)


# revision 15
# speedup vs baseline: 1.1177x; 1.1177x over previous
"""CenterLoss kernel for Trainium2, 8-core SPMD.

loss = mean_i( 0.5 * || x[i] - centers[labels[i]] ||^2 )

Sharding: data-parallel over the batch. Each of the 8 cores gets 128
samples (x rows + labels) and a replicated view of the full centers
table in its HBM. On-device, each core indirect-DMA-gathers only the
128 label-matched center rows (256 KB) — never touching the other
64 MB of the table — computes diff on the vector engine, a fused
square + row-sum on the scalar engine, and reduces across partitions
with a ones-vector matmul on the tensor engine. Each core emits one
partial sum; the host combines 8 scalars (the unshard step) and
applies the 0.5/N scaling.

Written in raw bacc (manual semaphores, no TileContext): the linear
dependency chain needs only a handful of sems, and skipping Tile's
exit barriers saves ~1.5us of epilogue. The NEFF epilogue's own drain
flushes the final output DMA, so no explicit wait on it is needed.

(Measured dead end, kept out: fusing the subtract into the gather via
the SDMA CCE ALU (compute_op=add on host-staged -x) is exact but ~2us
slower — RMW descriptor-gen and completion are slower, and the gather
then also waits on the x DMA's receipt.)
"""

import numpy as np

N_CORES = 8
BATCH = 1024
FEAT = 512
NUM_CLASSES = 32768
B_LOC = BATCH // N_CORES  # 128 == SBUF partition count

_compiled = None


def _build():
    import concourse.bacc as bacc
    import concourse.bass as bass
    import concourse.mybir as mybir

    f32 = mybir.dt.float32
    i32 = mybir.dt.int32

    nc = bacc.Bacc(
        "TRN2",
        target_bir_lowering=False,
        debug=False,
        enable_asserts=False,
        num_devices=N_CORES,
    )

    x_d = nc.dram_tensor("x", [B_LOC, FEAT], f32, kind="ExternalInput").ap()
    lab_d = nc.dram_tensor("labels", [B_LOC, 1], i32, kind="ExternalInput").ap()
    cen_d = nc.dram_tensor(
        "centers", [NUM_CLASSES, FEAT], f32, kind="ExternalInput"
    ).ap()
    out_d = nc.dram_tensor("out", [1, 1], f32, kind="ExternalOutput").ap()

    with (
        nc.sbuf_tensor("xt", [B_LOC, FEAT], f32) as xt,
        nc.sbuf_tensor("ct", [B_LOC, FEAT], f32) as ct,
        nc.sbuf_tensor("lab", [B_LOC, 1], i32) as lab,
        nc.sbuf_tensor("diff", [B_LOC, FEAT], f32) as diff,
        nc.sbuf_tensor("sq", [B_LOC, FEAT], f32) as sq,
        nc.sbuf_tensor("row", [B_LOC, 1], f32) as row,
        nc.sbuf_tensor("ones", [B_LOC, 1], f32) as ones,
        nc.sbuf_tensor("res", [1, 1], f32) as res,
        nc.psum_tensor("acc", [1, 1], f32) as acc,
        nc.semaphore("lsem") as lsem,
        nc.semaphore("xsem") as xsem,
        nc.semaphore("gsem") as gsem,
        nc.semaphore("osem") as osem,
        nc.semaphore("csem") as csem,
    ):
        # labels head the critical chain (labels -> gather); x load goes on
        # the scalar HWDGE queue so the two issue in parallel.
        nc.sync.dma_start(lab.ap(), lab_d).then_inc(lsem, 16)
        nc.scalar.dma_start(xt.ap(), x_d).then_inc(xsem, 16)
        nc.vector.memset(ones.ap(), 1.0)

        nc.gpsimd.wait_ge(lsem, 16)
        nc.gpsimd.indirect_dma_start(
            out=ct.ap(),
            out_offset=None,
            in_=cen_d,
            in_offset=bass.IndirectOffsetOnAxis(ap=lab.ap()[:, :1], axis=0),
        ).then_inc(gsem, 16)

        nc.vector.wait_ge(xsem, 16)
        nc.vector.wait_ge(gsem, 16)
        nc.vector.tensor_sub(out=diff.ap(), in0=xt.ap(), in1=ct.ap()).then_inc(
            csem, 1
        )

        # One ACT op: sq = diff*diff AND row = sum_free(sq)
        nc.scalar.wait_ge(csem, 1)
        nc.scalar.activation(
            out=sq.ap(),
            in_=diff.ap(),
            func=mybir.ActivationFunctionType.Square,
            accum_out=row.ap(),
        ).then_inc(csem, 1)

        # Partition reduce on PE: ones[128,1].T @ row[128,1] -> [1,1].
        # Keeps the output store a single 4B write (a [128,1] store is
        # 128 scattered 4B descriptors and stalls the final drain ~8us).
        # ones-read ordering: memset(DVE) precedes tensor_sub(DVE) in
        # program order, and the matmul waits on the csem chain the sub
        # started, so the memset is transitively ordered before it.
        nc.tensor.wait_ge(csem, 2)
        nc.tensor.matmul(
            out=acc.ap(), lhsT=ones.ap(), rhs=row.ap(), start=True, stop=True
        ).then_inc(csem, 1)

        nc.vector.wait_ge(csem, 3)
        nc.vector.tensor_copy(out=res.ap(), in_=acc.ap()).then_inc(csem, 1)

        nc.sync.wait_ge(csem, 4)
        # No wait on osem: the NEFF epilogue drain flushes this queue.
        nc.sync.dma_start(out_d, res.ap()).then_inc(osem, 16)

    nc.compile()
    return nc


def _get_compiled():
    global _compiled
    if _compiled is None:
        _compiled = _build()
    return _compiled


def _in_maps(x, labels, centers):
    xs = np.ascontiguousarray(np.asarray(x, dtype=np.float32)).reshape(
        N_CORES, B_LOC, FEAT
    )
    lab32 = np.ascontiguousarray(
        np.asarray(labels).astype(np.int32).reshape(N_CORES, B_LOC, 1)
    )
    cen = np.ascontiguousarray(np.asarray(centers, dtype=np.float32))
    return [
        {"x": xs[i], "labels": lab32[i], "centers": cen} for i in range(N_CORES)
    ]


def kernel(x, labels, centers):
    from concourse.bass_utils import run_bass_kernel_spmd

    nc = _get_compiled()
    res = run_bass_kernel_spmd(nc, _in_maps(x, labels, centers), list(range(N_CORES)))
    partials = np.array(
        [np.float64(r["out"].reshape(())) for r in res.results], dtype=np.float64
    )
    total = 0.5 * partials.sum() / BATCH
    return np.asarray(total, dtype=np.float32)
